# revision 15
# baseline (speedup 1.0000x reference)
"""Trainium2 Bass kernel for ConvMultiHeadAttention (N=16, L=1024, E=512, H=8).

Data-parallel over batch: 8 NeuronCores x 2 batches each. v3:
- HWDGE (sync) loads/stores; fp32 PE transposes writing fp16 PSUM (no DVE casts)
- S^T logits: one N=1024 fp16-PSUM matmul per head-side per key tile,
  row-tiling packs the head pair (rows 0:64 / 64:128) to co-issue on the PE
- softmax exp split: ScalarE LUT exp + DVE Schraudolph bit-trick exp
  (int16 y = x*2^10/ln2 + (15*1024-45), bitcast to fp16) on 25% of tiles
- denominator via ones column in AV (row 64); 1/d via Ln+Exp(-x) on ScalarE
- fpo-granular projections: pair pr of attention only needs fpo=pr of the
  Q/K projections, so softmax starts ~15us in; remaining prep work is
  emitted as PE filler under the Scalar-bound exp stream
"""

import numpy as np
import concourse.bass as bass
import concourse.mybir as mybir
import concourse.tile as tile
from contextlib import ExitStack
from concourse import bacc

P = 128
L = 1024
E = 512
H = 8
D = 64
NB = 2            # batches per core
TT = L // P       # 8 token tiles per batch
EPO = E // P      # 4 e-subtiles
NPAIR = H // 2    # 4 head pairs
FP32 = mybir.dt.float32
FP16 = mybir.dt.float16
I16 = mybir.dt.int16
AF = mybir.ActivationFunctionType
ALU = mybir.AluOpType

A16 = float(1024.0 / np.log(2.0))     # fp16 Schraudolph scale
B16 = float(15 * 1024 - 45)           # fp16 Schraudolph bias (C=45)


def trick_tile(b, pr, lt, side):
    # which exp tiles go to the DVE bit-trick exp (~25%)
    return side == 1 and (lt % 2 == 1)


def host_constants():
    ident = np.eye(P, dtype=np.float16)
    # sel2[p, 128*pair + j]: broadcasts denom rows to a head-pair's 128
    # stage partitions. head h denom lives at partition 32*(h%4), block h//4.
    sel2 = np.zeros((P, NPAIR * P), np.float32)
    for pr in range(NPAIR):
        h0, h1 = 2 * pr, 2 * pr + 1
        sel2[32 * (h0 % 4), pr * P:pr * P + D] = 1.0
        sel2[32 * (h1 % 4), pr * P + D:(pr + 1) * P] = 1.0
    return ident, sel2


def build(debug=False):
    nc = bacc.Bacc("TRN2", target_bir_lowering=False, debug=debug)
    q_d = nc.dram_tensor("q", [NB, L, E], FP32, kind="ExternalInput").ap()
    k_d = nc.dram_tensor("k", [NB, L, E], FP32, kind="ExternalInput").ap()
    v_d = nc.dram_tensor("v", [NB, L, E], FP32, kind="ExternalInput").ap()
    wq_d = nc.dram_tensor("Wq", [E, E], FP32, kind="ExternalInput").ap()
    wk_d = nc.dram_tensor("Wk", [E, E], FP32, kind="ExternalInput").ap()
    wv_d = nc.dram_tensor("Wv", [E, E], FP32, kind="ExternalInput").ap()
    wo_d = nc.dram_tensor("Wo", [E, E], FP32, kind="ExternalInput").ap()
    bo_d = nc.dram_tensor("bo_bcast", [P, E], FP32, kind="ExternalInput").ap()
    id_d = nc.dram_tensor("ident", [P, P], FP16, kind="ExternalInput").ap()
    sel_d = nc.dram_tensor("sel2", [P, NPAIR * P], FP32, kind="ExternalInput").ap()
    out_d = nc.dram_tensor("out", [NB, L, E], FP32, kind="ExternalOutput").ap()

    with tile.TileContext(nc) as tc, ExitStack() as ctx:
        consts = ctx.enter_context(tc.tile_pool(name="consts", bufs=1))
        wn_pool = ctx.enter_context(tc.tile_pool(name="wn", bufs=1))
        wt_pool = ctx.enter_context(tc.tile_pool(name="wt", bufs=1))
        xb_pool = ctx.enter_context(tc.tile_pool(name="xb", bufs=3))
        xt_pool = ctx.enter_context(tc.tile_pool(name="xt", bufs=4))
        qk_pool = ctx.enter_context(tc.tile_pool(name="qk", bufs=4))
        vh_pool = ctx.enter_context(tc.tile_pool(name="vh", bufs=2))
        st_pool = ctx.enter_context(tc.tile_pool(name="st", bufs=2))
        pt_pool = ctx.enter_context(tc.tile_pool(name="pp", bufs=22))
        dn_pool = ctx.enter_context(tc.tile_pool(name="dn", bufs=2))
        o_pool = ctx.enter_context(tc.tile_pool(name="oo", bufs=4))
        ps_mm = ctx.enter_context(tc.tile_pool(name="psmm", bufs=2, space="PSUM"))
        ps_s = ctx.enter_context(tc.tile_pool(name="pss", bufs=2, space="PSUM"))
        ps_o = ctx.enter_context(tc.tile_pool(name="pso", bufs=2, space="PSUM"))

        def emit_copy(out, in_, eng="v"):
            if eng == "s":
                nc.scalar.copy(out, in_)
            else:
                nc.vector.tensor_copy(out, in_)

        # ---- constants ----
        ident = consts.tile([P, P], FP16)
        nc.sync.dma_start(ident[:], id_d)
        sel = consts.tile([P, NPAIR * P], FP32)
        nc.sync.dma_start(sel[:], sel_d)
        bo_t = consts.tile([P, E], FP32)
        nc.sync.dma_start(bo_t[:], bo_d)

        # ---- weights: fp32 load, cast fp16, PE transpose -> wt[e_pi, epo, f] ----
        wts = {}
        wds = {"q": wq_d, "k": wk_d, "v": wv_d, "o": wo_d}

        def emit_weight(wname, ceng="v"):
            wn = wn_pool.tile([P, EPO, E], FP32, tag="wn")
            nc.sync.dma_start(wn[:], wds[wname].rearrange("(fo p) e -> p fo e", p=P))
            wnb = wn_pool.tile([P, EPO, E], FP16, tag="wnb")
            emit_copy(wnb[:], wn[:], ceng)
            wt = wt_pool.tile([P, EPO, E], FP16, tag=f"wt_{wname}",
                              name=f"wt_{wname}")
            for epo in range(EPO):
                ps = ps_mm.tile([P, E], FP16, tag="mm")
                for fo in range(EPO):
                    nc.tensor.transpose(
                        ps[:, fo * P:(fo + 1) * P],
                        wnb[:, fo, epo * P:(epo + 1) * P],
                        ident[:],
                    )
                if wname == "q":
                    # fold 1/sqrt(D) into Wq
                    nc.vector.tensor_scalar_mul(wt[:, epo, :], ps[:], 1.0 / np.sqrt(D))
                else:
                    emit_copy(wt[:, epo, :], ps[:], ceng)
            wts[wname] = wt

        xts = {}
        qkts = {}
        vhs = {}
        stages = {}
        denoms = {}

        def emit_xt(b, tname, x_d, eng="v"):
            xt = xt_pool.tile([P, EPO, L], FP16, tag="xt")
            for tt in range(TT):
                xb0 = xb_pool.tile([P, E], FP32, tag="xb")
                nc.sync.dma_start(xb0[:], x_d[b, tt * P:(tt + 1) * P, :])
                xb = xb_pool.tile([P, E], FP16, tag="xbh")
                emit_copy(xb[:], xb0[:], "v")
                ps = ps_mm.tile([P, E], FP16, tag="mm")
                for epo in range(EPO):
                    nc.tensor.transpose(
                        ps[:, epo * P:(epo + 1) * P],
                        xb[:, epo * P:(epo + 1) * P],
                        ident[:],
                    )
                emit_copy(
                    xt[:, :, tt * P:(tt + 1) * P],
                    ps[:].rearrange("p (epo t) -> p epo t", epo=EPO),
                    eng,
                )
            xts[(b, tname)] = xt

        def emit_qk_proj_fpo(b, tname, fpo, eng="v"):
            wt = wts[tname]
            xt = xts[(b, tname)]
            if (b, tname) not in qkts:
                qkts[(b, tname)] = qk_pool.tile([P, EPO, L], FP16, tag="ht",
                                                name=f"ht_{b}_{tname}")
            ht = qkts[(b, tname)]
            for tch in range(L // E):
                ps = ps_mm.tile([P, E], FP32, tag="mm")
                for epo in range(EPO):
                    nc.tensor.matmul(
                        ps[:],
                        wt[:, epo, fpo * P:(fpo + 1) * P],
                        xt[:, epo, tch * E:(tch + 1) * E],
                        start=(epo == 0),
                        stop=(epo == EPO - 1),
                    )
                emit_copy(ht[:, fpo, tch * E:(tch + 1) * E], ps[:], eng)

        def emit_v_proj(b, eng="v"):
            # vh natural [t_pi, tt, h, 65]; col 64 = ones (denominator trick)
            vh = vh_pool.tile([P, TT, H, D + 1], FP16, tag="vh")
            nc.vector.memset(vh[:], 1.0)
            wt = wts["v"]
            xt = xts[(b, "v")]
            for tt in range(TT):
                ps = ps_mm.tile([P, E], FP32, tag="mm")
                for epo in range(EPO):
                    nc.tensor.matmul(
                        ps[:],
                        xt[:, epo, tt * P:(tt + 1) * P],
                        wt[:, epo, :],
                        start=(epo == 0),
                        stop=(epo == EPO - 1),
                    )
                emit_copy(
                    vh[:, tt, :, 0:D],
                    ps[:].rearrange("p (h d) -> p h d", h=H),
                    eng,
                )
            vhs[b] = vh

        def setup_batch(b):
            stages[b] = st_pool.tile([P, EPO, L], FP16, tag="st",
                                     name=f"stage{b}")
            denoms[b] = dn_pool.tile([P, 2, L], FP32, tag="dn",
                                     name=f"denom{b}")
            nc.vector.memset(denoms[b][:], 1.0)

        def emit_s_exp(b, pr):
            # head pair (2pr, 2pr+1): single N=1024 fp16-PSUM matmul per
            # (side, key-tile); row tiling co-issues the pair on the PE
            qht, kht = qkts[(b, "q")], qkts[(b, "k")]
            pts = {0: [], 1: []}
            for lt in range(TT):
                pss = {0: ps_s.tile([P, L], FP32, tag="s", name="pssA"),
                       1: ps_s.tile([P, L], FP32, tag="s", name="pssB")}
                for ch in range(L // E):
                    for side in (0, 1):
                        r0 = side * D
                        nc.tensor.matmul(
                            pss[side][:, ch * E:(ch + 1) * E],
                            kht[r0:r0 + D, pr, lt * P:(lt + 1) * P],
                            qht[r0:r0 + D, pr, ch * E:(ch + 1) * E],
                            start=True,
                            stop=True,
                        )
                for side in (0, 1):
                    if trick_tile(b, pr, lt, side):
                        pt = pt_pool.tile([P, L], I16, tag="p", name="ptI")
                        nc.vector.tensor_scalar(
                            pt[:], pss[side][:], A16, B16, ALU.mult, ALU.add)
                        pts[side].append(pt[:].bitcast(FP16))
                    else:
                        pt = pt_pool.tile([P, L], FP16, tag="p", name="ptF")
                        nc.scalar.activation(pt[:], pss[side][:], AF.Exp)
                        pts[side].append(pt[:])
            return pts

        def emit_av(b, pr, pts):
            vh, stage, denom = vhs[b], stages[b], denoms[b]
            for side in (0, 1):
                h = 2 * pr + side
                r0 = side * D
                for ch in range(L // E):
                    pso = ps_o.tile([D + 1, E], FP32, tag="o")
                    for lt in range(TT):
                        nc.tensor.matmul(
                            pso[:],
                            vh[:, lt, h, :],
                            pts[side][lt][:, ch * E:(ch + 1) * E],
                            start=(lt == 0),
                            stop=(lt == TT - 1),
                        )
                    emit_copy(stage[r0:r0 + D, pr, ch * E:(ch + 1) * E],
                              pso[0:D, :])
                    emit_copy(
                        denom[32 * (h % 4):32 * (h % 4) + 1, h // 4,
                              ch * E:(ch + 1) * E],
                        pso[D:D + 1, :], "s")

        def emit_normalize(b, blk=None):
            stage, denom = stages[b], denoms[b]
            # 1/d = exp(-ln d) on ScalarE: Ln monolithic (one table switch),
            # Exp per block so the first pairs' broadcast starts earlier
            nc.scalar.activation(denom[:], denom[:], AF.Ln)
            for bk in range(2):
                nc.scalar.activation(denom[:, bk, :], denom[:, bk, :],
                                     AF.Exp, scale=-1.0)
            recip = denom
            for pr in range(NPAIR):
                for ch in range(L // E):
                    psb = ps_mm.tile([P, E], FP32, tag="mm")
                    nc.tensor.matmul(
                        psb[:],
                        sel[:, pr * P:(pr + 1) * P],
                        recip[:, pr // 2, ch * E:(ch + 1) * E],
                        start=True,
                        stop=True,
                    )
                    nc.vector.tensor_tensor(
                        stage[:, pr, ch * E:(ch + 1) * E],
                        psb[:],
                        stage[:, pr, ch * E:(ch + 1) * E],
                        ALU.mult,
                    )

        def emit_oproj(b):
            stage = stages[b]
            wt = wts["o"]
            for tt in range(TT):
                ps = ps_mm.tile([P, E], FP32, tag="mm")
                for fpo in range(EPO):
                    nc.tensor.matmul(
                        ps[:],
                        stage[:, fpo, tt * P:(tt + 1) * P],
                        wt[:, fpo, :],
                        start=(fpo == 0),
                        stop=(fpo == EPO - 1),
                    )
                ot = o_pool.tile([P, E], FP32, tag="ot")
                nc.vector.tensor_tensor(ot[:], ps[:], bo_t[:], ALU.add)
                nc.sync.dma_start(out_d[b, tt * P:(tt + 1) * P, :], ot[:])

        # ======== emission schedule ========
        # lead-in: just enough for pair (0,0): xt q/k + fpo0 projections
        emit_weight("q", "s")
        emit_weight("k", "s")
        emit_xt(0, "q", q_d, "s")
        emit_xt(0, "k", k_d, "s")
        emit_qk_proj_fpo(0, "q", 0, "s")
        emit_qk_proj_fpo(0, "k", 0, "s")
        setup_batch(0)

        # PE filler chunks run under the Scalar-bound exp stream.
        # filler[pr] must contain everything pair pr+1's S needs (fpo=pr+1)
        # and vh before the first AV.
        fill_b0 = {
            0: lambda: (emit_weight("v"),
                        emit_xt(0, "v", v_d),
                        emit_v_proj(0),
                        emit_qk_proj_fpo(0, "q", 1),
                        emit_qk_proj_fpo(0, "k", 1)),
            1: lambda: (emit_weight("o"),
                        emit_qk_proj_fpo(0, "q", 2),
                        emit_qk_proj_fpo(0, "k", 2)),
            2: lambda: (emit_qk_proj_fpo(0, "q", 3),
                        emit_qk_proj_fpo(0, "k", 3),
                        emit_xt(1, "q", q_d)),
            3: lambda: (emit_xt(1, "k", k_d),
                        emit_qk_proj_fpo(1, "q", 0),
                        emit_qk_proj_fpo(1, "k", 0)),
        }
        for pr in range(NPAIR):
            pts = emit_s_exp(0, pr)
            fill_b0[pr]()
            emit_av(0, pr, pts)

        # bridge: start b1 attention before normalizing/projecting b0 out
        setup_batch(1)
        emit_xt(1, "v", v_d)
        emit_v_proj(1)
        pts10 = emit_s_exp(1, 0)
        emit_normalize(0)
        emit_qk_proj_fpo(1, "q", 1)
        emit_qk_proj_fpo(1, "k", 1)
        emit_oproj(0)
        emit_av(1, 0, pts10)

        fill_b1 = {
            1: lambda: (emit_qk_proj_fpo(1, "q", 2),
                        emit_qk_proj_fpo(1, "k", 2)),
            2: lambda: (emit_qk_proj_fpo(1, "q", 3),
                        emit_qk_proj_fpo(1, "k", 3)),
            3: lambda: None,
        }
        for pr in range(1, NPAIR):
            pts = emit_s_exp(1, pr)
            fill_b1[pr]()
            emit_av(1, pr, pts)

        emit_normalize(1)
        emit_oproj(1)

    nc.compile()
    return nc


_COMPILED = None


def _get_compiled():
    global _COMPILED
    if _COMPILED is None:
        _COMPILED = build()
    return _COMPILED


def kernel(q, k, v, Wq, Wk, Wv, Wo, bo):
    import numpy as _np

    q = _np.ascontiguousarray(_np.asarray(q, dtype=_np.float32))
    k = _np.ascontiguousarray(_np.asarray(k, dtype=_np.float32))
    v = _np.ascontiguousarray(_np.asarray(v, dtype=_np.float32))
    Wq = _np.ascontiguousarray(_np.asarray(Wq, dtype=_np.float32))
    Wk = _np.ascontiguousarray(_np.asarray(Wk, dtype=_np.float32))
    Wv = _np.ascontiguousarray(_np.asarray(Wv, dtype=_np.float32))
    Wo = _np.ascontiguousarray(_np.asarray(Wo, dtype=_np.float32))
    bo = _np.asarray(bo, dtype=_np.float32)

    nc = _get_compiled()
    ident, sel2 = host_constants()
    bo_bcast = _np.ascontiguousarray(_np.broadcast_to(bo, (P, E)))
    n_cores = 8
    in_maps = []
    for c in range(n_cores):
        in_maps.append({
            "q": _np.ascontiguousarray(q[c * NB:(c + 1) * NB]),
            "k": _np.ascontiguousarray(k[c * NB:(c + 1) * NB]),
            "v": _np.ascontiguousarray(v[c * NB:(c + 1) * NB]),
            "Wq": Wq, "Wk": Wk, "Wv": Wv, "Wo": Wo,
            "bo_bcast": bo_bcast, "ident": ident, "sel2": sel2,
        })

    from concourse.bass_utils import run_bass_kernel_spmd
    res = run_bass_kernel_spmd(nc, in_maps, core_ids=list(range(n_cores)))
    out = _np.concatenate([res.results[c]["out"] for c in range(n_cores)], axis=0)
    return out.astype(_np.float32)
